# revision 9
# baseline (speedup 1.0000x reference)
"""GPT-J joint attention (B=1, S=2048, D=2048, H=16, HD=128) on 8 Trainium2
NeuronCores, tensor-parallel over heads (2 heads per core).

Per-core program (all matmuls bf16 inputs, fp32 PSUM accumulation):
  - QT/KT = W[qk]_shard @ hidden^T        ([hd, s] layout, per head)
  - RoPE applied via a rotation-matrix matmul + elementwise combine
  - V = hidden @ Wv_shard^T               ([s, hd] layout)
  - scores^T tiles = KT_tile^T . QT_block ([k, q] layout) -> exp -> causal mask
  - O^T accumulated as V_tile^T . P^T; softmax denominator: DVE lane sums of
    the exp tiles, merged, then ONE ones[128,128] matmul per (qb, head) that
    yields the denominator already broadcast across partitions; reciprocal
    (DVE) feeds the normalization multiply directly.
  - partial out = O^T{normalized}^T . Wo_shard^T, streamed to DRAM per block

Host side: shard/transpose/cast inputs, run SPMD on 8 cores, sum the 8
partial outputs (the tensor-parallel all-reduce equivalent).
"""
import sys

import numpy as np
import ml_dtypes

try:
    import concourse.bass as bass
except ImportError:  # pragma: no cover
    sys.path.insert(0, "/opt/trn_rl_repo")
    import concourse.bass as bass

import concourse.mybir as mybir
import concourse.tile as tile
from concourse.bass_utils import run_bass_kernel_spmd

BF16 = mybir.dt.bfloat16
F32 = mybir.dt.float32
NPBF16 = ml_dtypes.bfloat16

N_CORES = 8
S = 2048          # sequence length
D = 2048          # model dim
HD = 128          # head dim
NHC = 2           # heads per core
DC = NHC * HD     # shard width (256)
P = 128           # partitions
KD = D // P       # 16 contraction tiles over model dim
QBS = 512         # q-block size
NQB = S // QBS    # 4 q-blocks
NST = S // P      # 16 sequence tiles of 128
SCALE = 1.0 / float(np.sqrt(HD))

# ---------------------------------------------------------------------------
# Walrus's CoreV3 drain encoding accepts a single sem wait; Tile's tail drain
# carries one wait per logical proc. Split it into one drain per proc.
# ---------------------------------------------------------------------------


def _install_drain_split():
    if getattr(tile.TileContext, "_drain_split_installed", False):
        return
    from concourse.vector_clock import ScopedClock, VectorClock

    def _drain_and_barrier(self, tick_clock, wait_clock):
        full = tick_clock.global_clock
        n = len(full)
        for i in range(n):
            if full[i] <= 0:
                continue
            vec = [full[j] if j == i else 0 for j in range(n)]
            drain_inst = self.nc.sync.drain()
            wait_clock.add_sem_waits(
                drain_inst.ins, ScopedClock({None: VectorClock(vec)})
            )
        self.nc.all_engine_barrier()
        assert self.sems is not None
        popped = self.nc._tile_sem_poison_stack.pop()
        assert popped is self._sem_poison
        self.nc.clear_and_free_semaphores(list(self.sems.allocated().values()))
        self.nc.all_engine_barrier()

    tile.TileContext._drain_and_barrier = _drain_and_barrier
    tile.TileContext._drain_split_installed = True


def _split_excess_waits(nc, limit=1):
    """This walrus build rejects instructions carrying more than one sem wait
    (CoreV3 setupSyncWait: 'Too many sync wait commands'). Spill excess waits
    onto same-engine NOPs inserted just before the instruction — the engine
    executes them in queue order, so blocking semantics are unchanged."""
    ctr = 0
    for fn in nc.m.functions:
        for blk in fn.blocks:
            new_list = []
            for inst in blk.instructions:
                si = inst.sync_info
                if si is not None and len(si.on_wait) > limit:
                    waits = list(si.on_wait)
                    excess, keep = waits[:-limit], waits[-limit:]
                    for w in excess:
                        ctr += 1
                        nop = mybir.InstNoOp(
                            name=f"I-wsplit-{ctr}", text_hint="wait_split"
                        )
                        nop.engine = inst.engine
                        nop.sync_info = mybir.SyncInfo(on_wait=[w], on_update=[])
                        new_list.append(nop)
                    inst.sync_info = mybir.SyncInfo(
                        on_wait=keep, on_update=si.on_update
                    )
                new_list.append(inst)
            if len(new_list) != len(blk.instructions):
                blk.instructions[:] = new_list
    return ctr


def build_nc(split_waits=True):
    _install_drain_split()
    nc = bass.Bass()

    hT = nc.dram_tensor("hT", [D, S], BF16, kind="ExternalInput")
    wq = nc.dram_tensor("wq", [D, DC], BF16, kind="ExternalInput")
    wk = nc.dram_tensor("wk", [D, DC], BF16, kind="ExternalInput")
    wv = nc.dram_tensor("wv", [D, DC], BF16, kind="ExternalInput")
    wo = nc.dram_tensor("wo", [DC, D], BF16, kind="ExternalInput")
    ct = nc.dram_tensor("ct", [P, S], BF16, kind="ExternalInput")
    st = nc.dram_tensor("st", [P, S], BF16, kind="ExternalInput")
    rot = nc.dram_tensor("rot", [P, P], BF16, kind="ExternalInput")
    out = nc.dram_tensor("out", [S, D], BF16, kind="ExternalOutput")

    Exp = mybir.ActivationFunctionType.Exp
    Copy = mybir.ActivationFunctionType.Copy

    with tile.TileContext(nc) as tc:
        with (
            tc.tile_pool(name="const", bufs=1) as const,
            tc.tile_pool(name="acts", bufs=1) as acts,
            tc.tile_pool(name="work", bufs=2) as work,
            tc.tile_pool(name="ptpool", bufs=5) as ptpool,
            tc.tile_pool(name="outstage", bufs=3) as outstage,
            tc.tile_pool(name="ps_main", bufs=5, space="PSUM") as ps_main,
            tc.tile_pool(name="ps_ot", bufs=3, space="PSUM") as ps_ot,
        ):
            # ---- constants / weights into SBUF. wq + the first q-block of
            # hidden arrive as interleaved 4-kd chunks so the first projection
            # matmuls can start as soon as chunk 0 lands; everything else is
            # batched (HWDGE trigger sequencer costs ~0.4us per dma_start).
            wq_sb = const.tile([P, KD, DC], BF16)
            wk_sb = const.tile([P, KD, DC], BF16)
            wv_sb = const.tile([P, KD, DC], BF16)
            hT_sb = const.tile([P, KD, S], BF16)
            hT_r = hT.rearrange("(kd p) s -> p kd s", p=P)
            wq_r = wq.rearrange("(kd p) e -> p kd e", p=P)
            ct_sb = const.tile([P, S], BF16)
            st_sb = const.tile([P, S], BF16)
            rot_sb = const.tile([P, P], BF16)
            wo_sb = const.tile([P, NHC, D], BF16)
            wk_r = wk.rearrange("(kd p) e -> p kd e", p=P)
            for c2 in range(8):
                ksl = slice(2 * c2, 2 * c2 + 2)
                nc.sync.dma_start(out=wq_sb[:, ksl, :], in_=wq_r[:, ksl, :])
                nc.sync.dma_start(
                    out=hT_sb[:, ksl, 0:QBS], in_=hT_r[:, ksl, 0:QBS]
                )
                nc.sync.dma_start(out=wk_sb[:, ksl, :], in_=wk_r[:, ksl, :])
            nc.sync.dma_start(out=rot_sb, in_=rot[:, :])
            nc.sync.dma_start(out=ct_sb, in_=ct[:, :])
            nc.sync.dma_start(out=st_sb, in_=st[:, :])
            nc.sync.dma_start(
                out=wv_sb, in_=wv.rearrange("(kd p) e -> p kd e", p=P)
            )
            for qb in range(1, NQB):
                nc.sync.dma_start(
                    out=hT_sb[:, :, qb * QBS:(qb + 1) * QBS],
                    in_=hT_r[:, :, qb * QBS:(qb + 1) * QBS],
                )
            nc.sync.dma_start(
                out=wo_sb, in_=wo.rearrange("(k2 p) e -> p k2 e", p=P)
            )
            ones128 = const.tile([P, P], BF16)  # lhsT for partition-sum+bcast
            nc.vector.memset(ones128, 1.0)

            # persistent activations
            qt_sb = acts.tile([P, NHC, S], BF16)   # [hd, h, s] rotary-applied Q^T
            kt_sb = acts.tile([P, NHC, S], BF16)
            v_sb = acts.tile([P, NST, DC], BF16)   # [s%128, s//128, head*hd]
            otb_sb = acts.tile([P, NHC, S], BF16)  # normalized O^T per head

            # ---- emission thunks ------------------------------------------
            # The kernel is software-pipelined at the q-block level: the
            # attention kt-loop for block qb is ACT(exp)-bound on its own, so
            # the projection matmuls for block qb+1 and the out-projection for
            # block qb-1 are interleaved between its kt steps as PE filler.
            pending = []  # rope epilogues: (psum, dst_sb, h, qb)

            def flush_rope(keep=0):
                while len(pending) > keep:
                    ps, dst_sb, h, qb = pending.pop(0)
                    sl = slice(qb * QBS, (qb + 1) * QBS)
                    raw = work.tile([P, QBS], BF16, tag="raw")
                    nc.scalar.activation(raw, ps, Copy)
                    rps = ps_main.tile([P, QBS], F32, tag="mm")
                    nc.tensor.matmul(rps, lhsT=rot_sb, rhs=raw, start=True, stop=True)
                    t1 = work.tile([P, QBS], BF16, tag="t1")
                    t2 = work.tile([P, QBS], BF16, tag="t2")
                    # t1 on GpSimd (SBUF-only op) to keep the DVE free for
                    # the attention lane sums
                    nc.gpsimd.tensor_mul(t1, raw, ct_sb[:, sl])
                    nc.vector.tensor_mul(t2, rps, st_sb[:, sl])
                    nc.vector.tensor_add(dst_sb[:, h, sl], t1, t2)

            def project(w_sb, dst_sb, h, qb):
                flush_rope(keep=1)
                sl = slice(qb * QBS, (qb + 1) * QBS)
                ps = ps_main.tile([P, QBS], F32, name="proj_ps", tag="mm")
                for kd in range(KD):
                    nc.tensor.matmul(
                        ps,
                        lhsT=w_sb[:, kd, h * HD:(h + 1) * HD],
                        rhs=hT_sb[:, kd, sl],
                        start=(kd == 0),
                        stop=(kd == KD - 1),
                    )
                pending.append((ps, dst_sb, h, qb))

            def v_tile(st_idx):
                flush_rope(keep=1)
                ps = ps_main.tile([P, DC], F32, tag="mm")
                for kd in range(KD):
                    nc.tensor.matmul(
                        ps,
                        lhsT=hT_sb[:, kd, st_idx * P:(st_idx + 1) * P],
                        rhs=wv_sb[:, kd, :],
                        start=(kd == 0),
                        stop=(kd == KD - 1),
                    )
                nc.scalar.activation(v_sb[:, st_idx, :], ps, Copy)

            def proj_units(qb):
                units = []
                for h in range(NHC):
                    units.append(lambda h=h: project(wq_sb, qt_sb, h, qb))
                    units.append(lambda h=h: project(wk_sb, kt_sb, h, qb))
                for s4 in range(4):
                    units.append(lambda s4=s4: v_tile(qb * 4 + s4))
                return units

            def out_proj_unit(qb, s4, tail=False, dve_heavy=False):
                def thunk():
                    st_idx = qb * 4 + s4
                    ost = outstage.tile([P, D], BF16, tag="ost")
                    for eb in range(NQB):
                        ops = ps_main.tile([P, QBS], F32, name="ops", tag="mm")
                        for h in range(NHC):
                            nc.tensor.matmul(
                                ops,
                                lhsT=otb_sb[:, h, st_idx * P:(st_idx + 1) * P],
                                rhs=wo_sb[:, h, eb * QBS:(eb + 1) * QBS],
                                start=(h == 0),
                                stop=(h == NHC - 1),
                            )
                        osl = ost[:, eb * QBS:(eb + 1) * QBS]
                        on_act = (eb % 4 == 0) if dve_heavy else (eb % 2 == 0)
                        if on_act:
                            nc.scalar.activation(osl, ops, Copy)
                        else:
                            nc.vector.tensor_copy(osl, ops)
                        if tail and eb % 2 == 1:
                            # stream the tail out in halves: small final DMA
                            nc.sync.dma_start(
                                out=out[
                                    st_idx * P:(st_idx + 1) * P,
                                    (eb - 1) * QBS:(eb + 1) * QBS,
                                ],
                                in_=ost[:, (eb - 1) * QBS:(eb + 1) * QBS],
                            )
                    if not tail:
                        nc.sync.dma_start(
                            out=out[st_idx * P:(st_idx + 1) * P, :], in_=ost
                        )
                return thunk

            OT_LAG = 3  # P.V matmul trails the score matmul by OT_LAG kt
            # steps so its sem wait is already satisfied and LDWEIGHTS
            # pipelines.

            class AttnBlock:
                """Per-q-block attention state + step emitters.

                For qb > 0 the kt order is non-diagonal tiles first, then the
                diagonal tiles in ascending j: the diagonal score/exp/PV/lane
                work is then sliced to the causally-live columns [j*128:],
                and the denominator lanes are always initialized from a
                full-width (non-diagonal) tile. qb == 0 has only diagonal
                tiles and stays full-width.
                """

                def __init__(self, qb):
                    self.qb = qb
                    self.qsl = slice(qb * QBS, (qb + 1) * QBS)
                    kmax = (qb + 1) * 4
                    if qb == 0:
                        self.kt_order = list(range(4))
                    else:
                        self.kt_order = list(range(0, qb * 4)) + list(
                            range(qb * 4, kmax)
                        )
                    self.pts = {}
                    self.accs = [[None] * 2 for _ in range(NHC)]
                    self.ot_pss = [
                        ps_ot.tile([P, QBS], F32, name="ot_ps", tag="ps_ot")
                        for _ in range(NHC)
                    ]

                def w0(self, kt):
                    j = kt - self.qb * 4
                    if self.qb == 0 or j < 0:
                        return 0
                    return j * P

                def acc_pt(self, h, kt, pt):
                    # h0 lanes on DVE, h1 lanes on GpSimd: the two heads'
                    # denominator sums run on different engines in parallel
                    eng = nc.vector if h == 0 else nc.gpsimd
                    w0 = self.w0(kt)
                    lane = kt % 2
                    if self.accs[h][lane] is None:
                        assert w0 == 0  # lanes init from a full-width tile
                        acc = work.tile(
                            [P, QBS], BF16, name=f"za{h}_{lane}",
                            tag=f"za{h}_{lane}",
                        )
                        eng.tensor_copy(acc, pt)
                        self.accs[h][lane] = acc
                    else:
                        acc = self.accs[h][lane]
                        eng.tensor_add(acc[:, w0:], acc[:, w0:], pt[:, w0:])

                def pv_step(self, kt):
                    for h in range(NHC):
                        w0 = self.w0(kt)
                        nc.tensor.matmul(
                            self.ot_pss[h][:, w0:],
                            lhsT=v_sb[:, kt, h * HD:(h + 1) * HD],
                            rhs=self.pts[(h, kt)][:, w0:],
                            start=(kt == self.kt_order[0]),
                            stop=(kt == self.kt_order[-1]),
                            skip_group_check=True,
                        )

                def step(self, ki):
                    qb, kt = self.qb, self.kt_order[ki]
                    w0 = self.w0(kt)
                    for h in range(NHC):
                        sps = ps_main.tile([P, QBS], F32, tag="mm")
                        nc.tensor.matmul(
                            sps[:, w0:],
                            lhsT=kt_sb[:, h, kt * P:(kt + 1) * P],
                            rhs=qt_sb[:, h, qb * QBS + w0:(qb + 1) * QBS],
                            start=True,
                            stop=True,
                        )
                        pt = ptpool.tile([P, QBS], BF16, tag=f"pt{h}")
                        nc.scalar.activation(
                            pt[:, w0:], sps[:, w0:], Exp, scale=SCALE
                        )
                        j = kt - qb * 4
                        if j >= 0:  # diagonal tile: causal mask (on Pool)
                            if qb == 0:
                                w = min(P * (j + 1), QBS)
                                nc.gpsimd.affine_select(
                                    out=pt[:, 0:w],
                                    in_=pt[:, 0:w],
                                    compare_op=mybir.AluOpType.is_ge,
                                    fill=0.0,
                                    base=-kt * P,
                                    pattern=[[1, w]],
                                    channel_multiplier=-1,
                                )
                            else:
                                # only the 128-wide diagonal square is mixed;
                                # everything left of it was sliced away
                                nc.gpsimd.affine_select(
                                    out=pt[:, w0:w0 + P],
                                    in_=pt[:, w0:w0 + P],
                                    compare_op=mybir.AluOpType.is_ge,
                                    fill=0.0,
                                    base=0,
                                    pattern=[[1, P]],
                                    channel_multiplier=-1,
                                )
                        self.pts[(h, kt)] = pt
                        self.acc_pt(h, kt, pt)
                    if ki >= OT_LAG:
                        self.pv_step(self.kt_order[ki - OT_LAG])
                    if ki == len(self.kt_order) - 1:
                        for ki2 in range(max(len(self.kt_order) - OT_LAG, 0),
                                         len(self.kt_order)):
                            self.pv_step(self.kt_order[ki2])

                def den_chain(self):
                    for h in range(NHC):
                        eng = nc.vector if h == 0 else nc.gpsimd
                        l0, l1 = self.accs[h]
                        if l1 is not None:
                            eng.tensor_add(l0, l0, l1)
                        den_ps = ps_main.tile(
                            [P, QBS], F32, name="den_ps", tag="mm"
                        )
                        nc.tensor.matmul(
                            den_ps, lhsT=ones128, rhs=l0,
                            start=True, stop=True,
                        )
                        r_sb = work.tile([P, QBS], F32, tag=f"r{h}")
                        nc.vector.reciprocal(r_sb, den_ps)
                        nc.vector.tensor_mul(
                            otb_sb[:, h, self.qsl], self.ot_pss[h], r_sb
                        )

            # ---- pipelined emission ---------------------------------------
            # Block order 1,2,3,0: the final attention block is the smallest
            # (4 kt steps), so the kernel tail is short and engine-balanced.
            for u in proj_units(0):
                u()
            for u in proj_units(1):
                u()
            flush_rope()
            order = [1, 2, 3, 0] if NQB == 4 else list(range(NQB))
            prev = None
            for idx, qb in enumerate(order):
                blk = AttnBlock(qb)
                fillers = []
                nxt = order[idx + 1] if idx + 1 < len(order) else None
                if nxt is not None and nxt >= 2:
                    fillers += proj_units(nxt)
                if prev is not None:
                    # during the big qb=3 block the exp stream saturates the
                    # ACT engine; lean that window's psum->sbuf copies DVE
                    fillers += [
                        out_proj_unit(prev, s4, dve_heavy=(qb == 3))
                        for s4 in range(4)
                    ]
                ns = len(blk.kt_order)
                nf = len(fillers)
                fi = 0
                for ki in range(ns):
                    blk.step(ki)
                    tgt = (ki + 1) * nf // ns
                    while fi < tgt:
                        fillers[fi]()
                        fi += 1
                while fi < nf:
                    fillers[fi]()
                    fi += 1
                flush_rope()
                blk.den_chain()
                prev = qb

            for s4 in range(4):
                out_proj_unit(0, s4, tail=True)()
    if split_waits:
        _split_excess_waits(nc)
    return nc


_NC_CACHE = {}


def _get_nc():
    if "nc" not in _NC_CACHE:
        _NC_CACHE["nc"] = build_nc()
    return _NC_CACHE["nc"]


def _rotation_matrix_T():
    # rot(x)[2i] = -x[2i+1]; rot(x)[2i+1] = x[2i].  R[i,j] coefficient of x[j].
    R = np.zeros((HD, HD), np.float32)
    idx = np.arange(0, HD, 2)
    R[idx, idx + 1] = -1.0
    R[idx + 1, idx] = 1.0
    return np.ascontiguousarray(R.T)


def prepare_in_maps(hidden_states, sin, cos, Wq, Wk, Wv, Wo):
    hidden_states = np.asarray(hidden_states, dtype=np.float32)
    sin = np.asarray(sin, dtype=np.float32)
    cos = np.asarray(cos, dtype=np.float32)
    Wq = np.asarray(Wq, dtype=np.float32)
    Wk = np.asarray(Wk, dtype=np.float32)
    Wv = np.asarray(Wv, dtype=np.float32)
    Wo = np.asarray(Wo, dtype=np.float32)

    hT = np.ascontiguousarray(hidden_states[0].T).astype(NPBF16)
    ct = np.ascontiguousarray(np.repeat(cos, 2, axis=1).T).astype(NPBF16)
    st = np.ascontiguousarray(np.repeat(sin, 2, axis=1).T).astype(NPBF16)
    rot = _rotation_matrix_T().astype(NPBF16)

    in_maps = []
    for c in range(N_CORES):
        e0 = c * DC
        in_maps.append(
            {
                "hT": hT,
                "wq": np.ascontiguousarray(Wq[e0:e0 + DC, :].T).astype(NPBF16),
                "wk": np.ascontiguousarray(Wk[e0:e0 + DC, :].T).astype(NPBF16),
                "wv": np.ascontiguousarray(Wv[e0:e0 + DC, :].T).astype(NPBF16),
                "wo": np.ascontiguousarray(Wo[:, e0:e0 + DC].T).astype(NPBF16),
                "ct": ct,
                "st": st,
                "rot": rot,
            }
        )
    return in_maps


def kernel(hidden_states, attention_mask, sin, cos, Wq, Wk, Wv, Wo):
    in_maps = prepare_in_maps(hidden_states, sin, cos, Wq, Wk, Wv, Wo)
    nc = _get_nc()
    res = run_bass_kernel_spmd(nc, in_maps, list(range(N_CORES)))
    out = res.results[0]["out"].astype(np.float32)
    for c in range(1, N_CORES):
        out += res.results[c]["out"].astype(np.float32)
    return out[None]


# revision 13
# speedup vs baseline: 1.1129x; 1.1129x over previous
"""GPT-J joint attention (B=1, S=2048, D=2048, H=16, HD=128) on 8 Trainium2
NeuronCores, tensor-parallel over heads (2 heads per core).

Per-core program (all matmuls bf16 inputs, fp32 PSUM accumulation):
  - QT/KT = W[qk]_shard @ hidden^T        ([hd, s] layout, per head)
  - RoPE applied via a rotation-matrix matmul + elementwise combine
  - V = hidden @ Wv_shard^T               ([s, hd] layout)
  - scores^T tiles = KT_tile^T . QT_block ([k, q] layout) -> exp -> causal mask
  - O^T accumulated as V_tile^T . P^T; softmax denominator: DVE lane sums of
    the exp tiles, merged, then ONE ones[128,128] matmul per (qb, head) that
    yields the denominator already broadcast across partitions; reciprocal
    (DVE) feeds the normalization multiply directly.
  - partial out = O^T{normalized}^T . Wo_shard^T, streamed to DRAM per block

Host side: shard/transpose/cast inputs, run SPMD on 8 cores, sum the 8
partial outputs (the tensor-parallel all-reduce equivalent).
"""
import sys

import numpy as np
import ml_dtypes

try:
    import concourse.bass as bass
except ImportError:  # pragma: no cover
    sys.path.insert(0, "/opt/trn_rl_repo")
    import concourse.bass as bass

import concourse.mybir as mybir
import concourse.tile as tile
from concourse.bass_utils import run_bass_kernel_spmd

BF16 = mybir.dt.bfloat16
F32 = mybir.dt.float32
NPBF16 = ml_dtypes.bfloat16

N_CORES = 8
S = 2048          # sequence length
D = 2048          # model dim
HD = 128          # head dim
NHC = 2           # heads per core
DC = NHC * HD     # shard width (256)
P = 128           # partitions
KD = D // P       # 16 contraction tiles over model dim
QBS = 512         # q-block size
NQB = S // QBS    # 4 q-blocks
NST = S // P      # 16 sequence tiles of 128
SCALE = 1.0 / float(np.sqrt(HD))

# ---------------------------------------------------------------------------
# Walrus's CoreV3 drain encoding accepts a single sem wait; Tile's tail drain
# carries one wait per logical proc. Split it into one drain per proc.
# ---------------------------------------------------------------------------


def _install_drain_split():
    if getattr(tile.TileContext, "_drain_split_installed", False):
        return
    from concourse.vector_clock import ScopedClock, VectorClock

    def _drain_and_barrier(self, tick_clock, wait_clock):
        full = tick_clock.global_clock
        n = len(full)
        for i in range(n):
            if full[i] <= 0:
                continue
            vec = [full[j] if j == i else 0 for j in range(n)]
            drain_inst = self.nc.sync.drain()
            wait_clock.add_sem_waits(
                drain_inst.ins, ScopedClock({None: VectorClock(vec)})
            )
        self.nc.all_engine_barrier()
        assert self.sems is not None
        popped = self.nc._tile_sem_poison_stack.pop()
        assert popped is self._sem_poison
        self.nc.clear_and_free_semaphores(list(self.sems.allocated().values()))
        self.nc.all_engine_barrier()

    tile.TileContext._drain_and_barrier = _drain_and_barrier
    tile.TileContext._drain_split_installed = True


def _split_excess_waits(nc, limit=1):
    """This walrus build rejects instructions carrying more than one sem wait
    (CoreV3 setupSyncWait: 'Too many sync wait commands'). Spill excess waits
    onto same-engine NOPs inserted just before the instruction — the engine
    executes them in queue order, so blocking semantics are unchanged."""
    ctr = 0
    for fn in nc.m.functions:
        for blk in fn.blocks:
            new_list = []
            for inst in blk.instructions:
                si = inst.sync_info
                if si is not None and len(si.on_wait) > limit:
                    waits = list(si.on_wait)
                    excess, keep = waits[:-limit], waits[-limit:]
                    for w in excess:
                        ctr += 1
                        nop = mybir.InstNoOp(
                            name=f"I-wsplit-{ctr}", text_hint="wait_split"
                        )
                        nop.engine = inst.engine
                        nop.sync_info = mybir.SyncInfo(on_wait=[w], on_update=[])
                        new_list.append(nop)
                    inst.sync_info = mybir.SyncInfo(
                        on_wait=keep, on_update=si.on_update
                    )
                new_list.append(inst)
            if len(new_list) != len(blk.instructions):
                blk.instructions[:] = new_list
    return ctr


def build_nc(split_waits=True):
    _install_drain_split()
    nc = bass.Bass()

    hT = nc.dram_tensor("hT", [D, S], BF16, kind="ExternalInput")
    wq = nc.dram_tensor("wq", [D, DC], BF16, kind="ExternalInput")
    wk = nc.dram_tensor("wk", [D, DC], BF16, kind="ExternalInput")
    wv = nc.dram_tensor("wv", [D, DC], BF16, kind="ExternalInput")
    wo = nc.dram_tensor("wo", [DC, D], BF16, kind="ExternalInput")
    ct = nc.dram_tensor("ct", [P, S], BF16, kind="ExternalInput")
    st = nc.dram_tensor("st", [P, S], BF16, kind="ExternalInput")
    rot = nc.dram_tensor("rot", [P, P], BF16, kind="ExternalInput")
    out = nc.dram_tensor("out", [S, D], BF16, kind="ExternalOutput")

    Exp = mybir.ActivationFunctionType.Exp
    Copy = mybir.ActivationFunctionType.Copy
    Ln = mybir.ActivationFunctionType.Ln

    with tile.TileContext(nc) as tc:
        with (
            tc.tile_pool(name="const", bufs=1) as const,
            tc.tile_pool(name="acts", bufs=1) as acts,
            tc.tile_pool(name="work", bufs=2) as work,
            tc.tile_pool(name="ptpool", bufs=5) as ptpool,
            tc.tile_pool(name="outstage", bufs=3) as outstage,
            tc.tile_pool(name="ps_main", bufs=5, space="PSUM") as ps_main,
            tc.tile_pool(name="ps_ot", bufs=3, space="PSUM") as ps_ot,
        ):
            # ---- constants / weights into SBUF. wq + the first q-block of
            # hidden arrive as interleaved 4-kd chunks so the first projection
            # matmuls can start as soon as chunk 0 lands; everything else is
            # batched (HWDGE trigger sequencer costs ~0.4us per dma_start).
            wq_sb = const.tile([P, KD, DC], BF16)
            wk_sb = const.tile([P, KD, DC], BF16)
            wv_sb = const.tile([P, KD, DC], BF16)
            hT_sb = const.tile([P, KD, S], BF16)
            hT_r = hT.rearrange("(kd p) s -> p kd s", p=P)
            wq_r = wq.rearrange("(kd p) e -> p kd e", p=P)
            ct_sb = const.tile([P, S], BF16)
            st_sb = const.tile([P, S], BF16)
            rot_sb = const.tile([P, P], BF16)
            wo_sb = const.tile([P, NHC, D], BF16)
            wk_r = wk.rearrange("(kd p) e -> p kd e", p=P)
            for c2 in range(8):
                ksl = slice(2 * c2, 2 * c2 + 2)
                nc.sync.dma_start(out=wq_sb[:, ksl, :], in_=wq_r[:, ksl, :])
                nc.sync.dma_start(
                    out=hT_sb[:, ksl, 0:QBS], in_=hT_r[:, ksl, 0:QBS]
                )
                nc.sync.dma_start(out=wk_sb[:, ksl, :], in_=wk_r[:, ksl, :])
            nc.sync.dma_start(out=rot_sb, in_=rot[:, :])
            nc.sync.dma_start(out=ct_sb, in_=ct[:, :])
            nc.sync.dma_start(out=st_sb, in_=st[:, :])
            nc.sync.dma_start(
                out=wv_sb, in_=wv.rearrange("(kd p) e -> p kd e", p=P)
            )
            for qb in range(1, NQB):
                nc.sync.dma_start(
                    out=hT_sb[:, :, qb * QBS:(qb + 1) * QBS],
                    in_=hT_r[:, :, qb * QBS:(qb + 1) * QBS],
                )
            nc.sync.dma_start(
                out=wo_sb, in_=wo.rearrange("(k2 p) e -> p k2 e", p=P)
            )
            ones128 = const.tile([P, P], BF16)  # lhsT for partition-sum+bcast
            nc.vector.memset(ones128, 1.0)

            # persistent activations
            qt_sb = acts.tile([P, NHC, S], BF16)   # [hd, h, s] rotary-applied Q^T
            kt_sb = acts.tile([P, NHC, S], BF16)
            v_sb = acts.tile([P, NST, DC], BF16)   # [s%128, s//128, head*hd]
            otb_sb = acts.tile([P, NHC, S], BF16)  # normalized O^T per head

            # ---- emission thunks ------------------------------------------
            # The kernel is software-pipelined at the q-block level: the
            # attention kt-loop for block qb is ACT(exp)-bound on its own, so
            # the projection matmuls for block qb+1 and the out-projection for
            # block qb-1 are interleaved between its kt steps as PE filler.
            pending = []  # rope epilogues: (psum, dst_sb, h, qb)

            def flush_rope(keep=0):
                while len(pending) > keep:
                    ps, dst_sb, h, qb = pending.pop(0)
                    sl = slice(qb * QBS, (qb + 1) * QBS)
                    raw = work.tile([P, QBS], BF16, tag="raw")
                    nc.scalar.activation(raw, ps, Copy)
                    rps = ps_main.tile([P, QBS], F32, tag="mm")
                    nc.tensor.matmul(rps, lhsT=rot_sb, rhs=raw, start=True, stop=True)
                    t1 = work.tile([P, QBS], BF16, tag="t1")
                    t2 = work.tile([P, QBS], BF16, tag="t2")
                    nc.vector.tensor_mul(t1, raw, ct_sb[:, sl])
                    nc.vector.tensor_mul(t2, rps, st_sb[:, sl])
                    nc.vector.tensor_add(dst_sb[:, h, sl], t1, t2)

            def project(w_sb, dst_sb, h, qb):
                flush_rope(keep=1)
                sl = slice(qb * QBS, (qb + 1) * QBS)
                ps = ps_main.tile([P, QBS], F32, name="proj_ps", tag="mm")
                for kd in range(KD):
                    nc.tensor.matmul(
                        ps,
                        lhsT=w_sb[:, kd, h * HD:(h + 1) * HD],
                        rhs=hT_sb[:, kd, sl],
                        start=(kd == 0),
                        stop=(kd == KD - 1),
                    )
                pending.append((ps, dst_sb, h, qb))

            def v_tile(st_idx):
                flush_rope(keep=1)
                ps = ps_main.tile([P, DC], F32, tag="mm")
                for kd in range(KD):
                    nc.tensor.matmul(
                        ps,
                        lhsT=hT_sb[:, kd, st_idx * P:(st_idx + 1) * P],
                        rhs=wv_sb[:, kd, :],
                        start=(kd == 0),
                        stop=(kd == KD - 1),
                    )
                nc.scalar.activation(v_sb[:, st_idx, :], ps, Copy)

            def proj_units(qb):
                units = []
                for h in range(NHC):
                    units.append(lambda h=h: project(wq_sb, qt_sb, h, qb))
                    units.append(lambda h=h: project(wk_sb, kt_sb, h, qb))
                for s4 in range(4):
                    units.append(lambda s4=s4: v_tile(qb * 4 + s4))
                return units

            def out_proj_unit(qb, s4, tail=False, dve_heavy=False):
                def thunk():
                    st_idx = qb * 4 + s4
                    ost = outstage.tile([P, D], BF16, tag="ost")
                    for eb in range(NQB):
                        ops = ps_main.tile([P, QBS], F32, name="ops", tag="mm")
                        for h in range(NHC):
                            nc.tensor.matmul(
                                ops,
                                lhsT=otb_sb[:, h, st_idx * P:(st_idx + 1) * P],
                                rhs=wo_sb[:, h, eb * QBS:(eb + 1) * QBS],
                                start=(h == 0),
                                stop=(h == NHC - 1),
                            )
                        osl = ost[:, eb * QBS:(eb + 1) * QBS]
                        on_act = (eb % 4 == 0) if dve_heavy else (eb % 2 == 0)
                        if on_act:
                            nc.scalar.activation(osl, ops, Copy)
                        else:
                            nc.vector.tensor_copy(osl, ops)
                        if tail and eb % 2 == 1:
                            # stream the tail out in halves: small final DMA
                            nc.sync.dma_start(
                                out=out[
                                    st_idx * P:(st_idx + 1) * P,
                                    (eb - 1) * QBS:(eb + 1) * QBS,
                                ],
                                in_=ost[:, (eb - 1) * QBS:(eb + 1) * QBS],
                            )
                    if not tail:
                        nc.sync.dma_start(
                            out=out[st_idx * P:(st_idx + 1) * P, :], in_=ost
                        )
                return thunk

            OT_LAG = 3  # P.V matmul trails the score matmul by OT_LAG kt
            # steps so its sem wait is already satisfied and LDWEIGHTS
            # pipelines.

            class AttnBlock:
                """Per-q-block attention state + step emitters.

                For qb > 0 the kt order is non-diagonal tiles first, then the
                diagonal tiles in ascending j: the diagonal score/exp/PV/lane
                work is then sliced to the causally-live columns [j*128:],
                and the denominator lanes are always initialized from a
                full-width (non-diagonal) tile. qb == 0 has only diagonal
                tiles and stays full-width.
                """

                def __init__(self, qb):
                    self.qb = qb
                    self.qsl = slice(qb * QBS, (qb + 1) * QBS)
                    kmax = (qb + 1) * 4
                    if qb == 0:
                        self.kt_order = list(range(4))
                    else:
                        self.kt_order = list(range(0, qb * 4)) + list(
                            range(qb * 4, kmax)
                        )
                    self.pts = {}
                    self.accs = [[None] * 2 for _ in range(NHC)]
                    self.ot_pss = [
                        ps_ot.tile([P, QBS], F32, name="ot_ps", tag="ps_ot")
                        for _ in range(NHC)
                    ]

                def w0(self, kt):
                    j = kt - self.qb * 4
                    if self.qb == 0 or j < 0:
                        return 0
                    return j * P

                def acc_pt(self, h, kt, pt):
                    # h0 lane sums on DVE, h1 lane sums on GpSimd (the Pool
                    # engine is otherwise idle); inits stay on the faster DVE
                    w0 = self.w0(kt)
                    lane = kt % 2
                    if self.accs[h][lane] is None:
                        assert w0 == 0  # lanes init from a full-width tile
                        acc = work.tile(
                            [P, QBS], BF16, name=f"za{h}_{lane}",
                            tag=f"za{h}_{lane}",
                        )
                        nc.vector.tensor_copy(acc, pt)
                        self.accs[h][lane] = acc
                    else:
                        acc = self.accs[h][lane]
                        eng = nc.vector if h == 0 else nc.gpsimd
                        eng.tensor_add(acc[:, w0:], acc[:, w0:], pt[:, w0:])

                def pv_step(self, kt):
                    for h in range(NHC):
                        w0 = self.w0(kt)
                        nc.tensor.matmul(
                            self.ot_pss[h][:, w0:],
                            lhsT=v_sb[:, kt, h * HD:(h + 1) * HD],
                            rhs=self.pts[(h, kt)][:, w0:],
                            start=(kt == self.kt_order[0]),
                            stop=(kt == self.kt_order[-1]),
                            skip_group_check=True,
                        )

                def step(self, ki):
                    qb, kt = self.qb, self.kt_order[ki]
                    w0 = self.w0(kt)
                    for h in range(NHC):
                        sps = ps_main.tile([P, QBS], F32, tag="mm")
                        nc.tensor.matmul(
                            sps[:, w0:],
                            lhsT=kt_sb[:, h, kt * P:(kt + 1) * P],
                            rhs=qt_sb[:, h, qb * QBS + w0:(qb + 1) * QBS],
                            start=True,
                            stop=True,
                        )
                        pt = ptpool.tile([P, QBS], BF16, tag=f"pt{h}")
                        nc.scalar.activation(
                            pt[:, w0:], sps[:, w0:], Exp, scale=SCALE
                        )
                        j = kt - qb * 4
                        if j >= 0:  # diagonal tile: causal mask (on Pool)
                            if qb == 0:
                                w = min(P * (j + 1), QBS)
                                nc.gpsimd.affine_select(
                                    out=pt[:, 0:w],
                                    in_=pt[:, 0:w],
                                    compare_op=mybir.AluOpType.is_ge,
                                    fill=0.0,
                                    base=-kt * P,
                                    pattern=[[1, w]],
                                    channel_multiplier=-1,
                                )
                            else:
                                # only the 128-wide diagonal square is mixed;
                                # everything left of it was sliced away
                                nc.gpsimd.affine_select(
                                    out=pt[:, w0:w0 + P],
                                    in_=pt[:, w0:w0 + P],
                                    compare_op=mybir.AluOpType.is_ge,
                                    fill=0.0,
                                    base=0,
                                    pattern=[[1, P]],
                                    channel_multiplier=-1,
                                )
                        self.pts[(h, kt)] = pt
                        self.acc_pt(h, kt, pt)
                    if ki >= OT_LAG:
                        self.pv_step(self.kt_order[ki - OT_LAG])
                    if ki == len(self.kt_order) - 1:
                        for ki2 in range(max(len(self.kt_order) - OT_LAG, 0),
                                         len(self.kt_order)):
                            self.pv_step(self.kt_order[ki2])

                def den_chain(self):
                    for h in range(NHC):
                        l0, l1 = self.accs[h]
                        if l1 is not None:
                            nc.vector.tensor_add(l0, l0, l1)
                        den_ps = ps_main.tile(
                            [P, QBS], F32, name="den_ps", tag="mm"
                        )
                        nc.tensor.matmul(
                            den_ps, lhsT=ones128, rhs=l0,
                            start=True, stop=True,
                        )
                        # 1/den = exp(-ln(den)) — both funcs live in the SAME
                        # ACT table as the attention Exp (no table reload),
                        # and the DVE reciprocal macro (3.3us!) is avoided.
                        lt = work.tile([P, QBS], F32, tag=f"lt{h}")
                        nc.scalar.activation(lt, den_ps, Ln)
                        r_sb = work.tile([P, QBS], BF16, tag=f"r{h}")
                        nc.scalar.activation(r_sb, lt, Exp, scale=-1.0)
                        nc.vector.tensor_mul(
                            otb_sb[:, h, self.qsl], self.ot_pss[h], r_sb
                        )

            # ---- pipelined emission ---------------------------------------
            # Block order 1,2,3,0: the final attention block is the smallest
            # (4 kt steps), so the kernel tail is short and engine-balanced.
            for u in proj_units(0):
                u()
            for u in proj_units(1):
                u()
            flush_rope()
            order = [1, 2, 3, 0] if NQB == 4 else list(range(NQB))
            prev = None
            for idx, qb in enumerate(order):
                blk = AttnBlock(qb)
                fillers = []
                nxt = order[idx + 1] if idx + 1 < len(order) else None
                if nxt is not None and nxt >= 2:
                    fillers += proj_units(nxt)
                if prev is not None:
                    # during the big qb=3 block the exp stream saturates the
                    # ACT engine; lean that window's psum->sbuf copies DVE
                    fillers += [
                        out_proj_unit(prev, s4, dve_heavy=(qb == 3))
                        for s4 in range(4)
                    ]
                ns = len(blk.kt_order)
                nf = len(fillers)
                fi = 0
                for ki in range(ns):
                    blk.step(ki)
                    tgt = (ki + 1) * nf // ns
                    while fi < tgt:
                        fillers[fi]()
                        fi += 1
                while fi < nf:
                    fillers[fi]()
                    fi += 1
                flush_rope()
                blk.den_chain()
                prev = qb

            for s4 in range(4):
                out_proj_unit(0, s4, tail=True)()
    if split_waits:
        _split_excess_waits(nc)
    return nc


_NC_CACHE = {}


def _get_nc():
    if "nc" not in _NC_CACHE:
        _NC_CACHE["nc"] = build_nc()
    return _NC_CACHE["nc"]


def _rotation_matrix_T():
    # rot(x)[2i] = -x[2i+1]; rot(x)[2i+1] = x[2i].  R[i,j] coefficient of x[j].
    R = np.zeros((HD, HD), np.float32)
    idx = np.arange(0, HD, 2)
    R[idx, idx + 1] = -1.0
    R[idx + 1, idx] = 1.0
    return np.ascontiguousarray(R.T)


def prepare_in_maps(hidden_states, sin, cos, Wq, Wk, Wv, Wo):
    hidden_states = np.asarray(hidden_states, dtype=np.float32)
    sin = np.asarray(sin, dtype=np.float32)
    cos = np.asarray(cos, dtype=np.float32)
    Wq = np.asarray(Wq, dtype=np.float32)
    Wk = np.asarray(Wk, dtype=np.float32)
    Wv = np.asarray(Wv, dtype=np.float32)
    Wo = np.asarray(Wo, dtype=np.float32)

    hT = np.ascontiguousarray(hidden_states[0].T).astype(NPBF16)
    ct = np.ascontiguousarray(np.repeat(cos, 2, axis=1).T).astype(NPBF16)
    st = np.ascontiguousarray(np.repeat(sin, 2, axis=1).T).astype(NPBF16)
    rot = _rotation_matrix_T().astype(NPBF16)

    in_maps = []
    for c in range(N_CORES):
        e0 = c * DC
        in_maps.append(
            {
                "hT": hT,
                "wq": np.ascontiguousarray(Wq[e0:e0 + DC, :].T).astype(NPBF16),
                "wk": np.ascontiguousarray(Wk[e0:e0 + DC, :].T).astype(NPBF16),
                "wv": np.ascontiguousarray(Wv[e0:e0 + DC, :].T).astype(NPBF16),
                "wo": np.ascontiguousarray(Wo[:, e0:e0 + DC].T).astype(NPBF16),
                "ct": ct,
                "st": st,
                "rot": rot,
            }
        )
    return in_maps


def kernel(hidden_states, attention_mask, sin, cos, Wq, Wk, Wv, Wo):
    in_maps = prepare_in_maps(hidden_states, sin, cos, Wq, Wk, Wv, Wo)
    nc = _get_nc()
    res = run_bass_kernel_spmd(nc, in_maps, list(range(N_CORES)))
    out = res.results[0]["out"].astype(np.float32)
    for c in range(1, N_CORES):
        out += res.results[c]["out"].astype(np.float32)
    return out[None]


# revision 18
# speedup vs baseline: 1.1134x; 1.0005x over previous
"""GPT-J joint attention (B=1, S=2048, D=2048, H=16, HD=128) on 8 Trainium2
NeuronCores, tensor-parallel over heads (2 heads per core).

Per-core program (all matmuls bf16 inputs, fp32 PSUM accumulation):
  - QT/KT = W[qk]_shard @ hidden^T        ([hd, s] layout, per head)
  - RoPE applied via a rotation-matrix matmul + elementwise combine
  - V = hidden @ Wv_shard^T               ([s, hd] layout)
  - scores^T tiles = KT_tile^T . QT_block ([k, q] layout) -> exp -> causal mask
  - O^T accumulated as V_tile^T . P^T; softmax denominator: DVE lane sums of
    the exp tiles, merged, then ONE ones[128,128] matmul per (qb, head) that
    yields the denominator already broadcast across partitions; reciprocal
    (DVE) feeds the normalization multiply directly.
  - partial out = O^T{normalized}^T . Wo_shard^T, streamed to DRAM per block

Host side: shard/transpose/cast inputs, run SPMD on 8 cores, sum the 8
partial outputs (the tensor-parallel all-reduce equivalent).
"""
import sys

import numpy as np
import ml_dtypes

try:
    import concourse.bass as bass
except ImportError:  # pragma: no cover
    sys.path.insert(0, "/opt/trn_rl_repo")
    import concourse.bass as bass

import concourse.mybir as mybir
import concourse.tile as tile
from concourse.bass_utils import run_bass_kernel_spmd

BF16 = mybir.dt.bfloat16
F32 = mybir.dt.float32
NPBF16 = ml_dtypes.bfloat16

N_CORES = 8
S = 2048          # sequence length
D = 2048          # model dim
HD = 128          # head dim
NHC = 2           # heads per core
DC = NHC * HD     # shard width (256)
P = 128           # partitions
KD = D // P       # 16 contraction tiles over model dim
QBS = 512         # q-block size
NQB = S // QBS    # 4 q-blocks
NST = S // P      # 16 sequence tiles of 128
SCALE = 1.0 / float(np.sqrt(HD))

# ---------------------------------------------------------------------------
# Walrus's CoreV3 drain encoding accepts a single sem wait; Tile's tail drain
# carries one wait per logical proc. Split it into one drain per proc.
# ---------------------------------------------------------------------------


def _install_drain_split():
    if getattr(tile.TileContext, "_drain_split_installed", False):
        return
    from concourse.vector_clock import ScopedClock, VectorClock

    def _drain_and_barrier(self, tick_clock, wait_clock):
        full = tick_clock.global_clock
        n = len(full)
        for i in range(n):
            if full[i] <= 0:
                continue
            vec = [full[j] if j == i else 0 for j in range(n)]
            drain_inst = self.nc.sync.drain()
            wait_clock.add_sem_waits(
                drain_inst.ins, ScopedClock({None: VectorClock(vec)})
            )
        self.nc.all_engine_barrier()
        assert self.sems is not None
        popped = self.nc._tile_sem_poison_stack.pop()
        assert popped is self._sem_poison
        self.nc.clear_and_free_semaphores(list(self.sems.allocated().values()))
        self.nc.all_engine_barrier()

    tile.TileContext._drain_and_barrier = _drain_and_barrier
    tile.TileContext._drain_split_installed = True


def _split_excess_waits(nc, limit=1):
    """This walrus build rejects instructions carrying more than one sem wait
    (CoreV3 setupSyncWait: 'Too many sync wait commands'). Spill excess waits
    onto same-engine NOPs inserted just before the instruction — the engine
    executes them in queue order, so blocking semantics are unchanged."""
    ctr = 0
    for fn in nc.m.functions:
        for blk in fn.blocks:
            new_list = []
            for inst in blk.instructions:
                si = inst.sync_info
                if si is not None and len(si.on_wait) > limit:
                    waits = list(si.on_wait)
                    excess, keep = waits[:-limit], waits[-limit:]
                    for w in excess:
                        ctr += 1
                        nop = mybir.InstNoOp(
                            name=f"I-wsplit-{ctr}", text_hint="wait_split"
                        )
                        nop.engine = inst.engine
                        nop.sync_info = mybir.SyncInfo(on_wait=[w], on_update=[])
                        new_list.append(nop)
                    inst.sync_info = mybir.SyncInfo(
                        on_wait=keep, on_update=si.on_update
                    )
                new_list.append(inst)
            if len(new_list) != len(blk.instructions):
                blk.instructions[:] = new_list
    return ctr


def build_nc(split_waits=True):
    _install_drain_split()
    nc = bass.Bass()

    hT = nc.dram_tensor("hT", [D, S], BF16, kind="ExternalInput")
    wq = nc.dram_tensor("wq", [D, DC], BF16, kind="ExternalInput")
    wk = nc.dram_tensor("wk", [D, DC], BF16, kind="ExternalInput")
    wv = nc.dram_tensor("wv", [D, DC], BF16, kind="ExternalInput")
    wo = nc.dram_tensor("wo", [DC, D], BF16, kind="ExternalInput")
    ct = nc.dram_tensor("ct", [P, S], BF16, kind="ExternalInput")
    st = nc.dram_tensor("st", [P, S], BF16, kind="ExternalInput")
    rot = nc.dram_tensor("rot", [P, P], BF16, kind="ExternalInput")
    out = nc.dram_tensor("out", [S, D], BF16, kind="ExternalOutput")

    Exp = mybir.ActivationFunctionType.Exp
    Copy = mybir.ActivationFunctionType.Copy
    Ln = mybir.ActivationFunctionType.Ln

    with tile.TileContext(nc) as tc:
        with (
            tc.tile_pool(name="const", bufs=1) as const,
            tc.tile_pool(name="acts", bufs=1) as acts,
            tc.tile_pool(name="work", bufs=2) as work,
            tc.tile_pool(name="ptpool", bufs=5) as ptpool,
            tc.tile_pool(name="outstage", bufs=3) as outstage,
            tc.tile_pool(name="ps_main", bufs=3, space="PSUM") as ps_main,
            tc.tile_pool(name="ps_ot", bufs=2, space="PSUM") as ps_ot,
        ):
            # ---- constants / weights into SBUF. wq + the first q-block of
            # hidden arrive as interleaved 4-kd chunks so the first projection
            # matmuls can start as soon as chunk 0 lands; everything else is
            # batched (HWDGE trigger sequencer costs ~0.4us per dma_start).
            wq_sb = const.tile([P, KD, DC], BF16)
            wk_sb = const.tile([P, KD, DC], BF16)
            wv_sb = const.tile([P, KD, DC], BF16)
            hT_sb = const.tile([P, KD, S], BF16)
            hT_r = hT.rearrange("(kd p) s -> p kd s", p=P)
            wq_r = wq.rearrange("(kd p) e -> p kd e", p=P)
            ct_sb = const.tile([P, S], BF16)
            st_sb = const.tile([P, S], BF16)
            rot_sb = const.tile([P, P], BF16)
            wo_sb = const.tile([P, NHC, D], BF16)
            wk_r = wk.rearrange("(kd p) e -> p kd e", p=P)
            # arrival order matches first consumption: Q-proj eats (wq, hT)
            # kd-pair by kd-pair, K-proj follows ~3.4us later
            for c2 in range(8):
                ksl = slice(2 * c2, 2 * c2 + 2)
                nc.sync.dma_start(out=wq_sb[:, ksl, :], in_=wq_r[:, ksl, :])
                nc.sync.dma_start(
                    out=hT_sb[:, ksl, 0:QBS], in_=hT_r[:, ksl, 0:QBS]
                )
            for c4 in range(2):
                ksl = slice(8 * c4, 8 * c4 + 8)
                nc.sync.dma_start(out=wk_sb[:, ksl, :], in_=wk_r[:, ksl, :])
            nc.sync.dma_start(out=rot_sb, in_=rot[:, :])
            nc.sync.dma_start(out=ct_sb, in_=ct[:, :])
            nc.sync.dma_start(out=st_sb, in_=st[:, :])
            nc.sync.dma_start(
                out=wv_sb, in_=wv.rearrange("(kd p) e -> p kd e", p=P)
            )
            for qb in range(1, NQB):
                nc.sync.dma_start(
                    out=hT_sb[:, :, qb * QBS:(qb + 1) * QBS],
                    in_=hT_r[:, :, qb * QBS:(qb + 1) * QBS],
                )
            nc.sync.dma_start(
                out=wo_sb, in_=wo.rearrange("(k2 p) e -> p k2 e", p=P)
            )
            ones128 = const.tile([P, P], BF16)  # lhsT for partition-sum+bcast
            nc.vector.memset(ones128, 1.0)

            # persistent activations
            qt_sb = acts.tile([P, NHC, S], BF16)   # [hd, h, s] rotary-applied Q^T
            kt_sb = acts.tile([P, NHC, S], BF16)
            v_sb = acts.tile([P, NST, DC], BF16)   # [s%128, s//128, head*hd]
            otb_sb = acts.tile([P, NHC, S], BF16)  # normalized O^T per head

            # ---- emission thunks ------------------------------------------
            # The kernel is software-pipelined at the q-block level: the
            # attention kt-loop for block qb is ACT(exp)-bound on its own, so
            # the projection matmuls for block qb+1 and the out-projection for
            # block qb-1 are interleaved between its kt steps as PE filler.
            pending = []  # rope epilogues: (psum, dst_sb, h, qb)

            def flush_rope(keep=0):
                while len(pending) > keep:
                    ps, dst_sb, h, qb = pending.pop(0)
                    sl = slice(qb * QBS, (qb + 1) * QBS)
                    raw = work.tile([P, QBS], BF16, tag="raw")
                    nc.scalar.activation(raw, ps, Copy)
                    rps = ps_main.tile([P, QBS], F32, tag="mm")
                    nc.tensor.matmul(rps, lhsT=rot_sb, rhs=raw, start=True, stop=True)
                    t1 = work.tile([P, QBS], BF16, tag="t1")
                    t2 = work.tile([P, QBS], BF16, tag="t2")
                    nc.vector.tensor_mul(t1, raw, ct_sb[:, sl])
                    nc.vector.tensor_mul(t2, rps, st_sb[:, sl])
                    nc.vector.tensor_add(dst_sb[:, h, sl], t1, t2)

            def project(w_sb, dst_sb, h, qb):
                flush_rope(keep=1)
                sl = slice(qb * QBS, (qb + 1) * QBS)
                ps = ps_main.tile([P, QBS], F32, name="proj_ps", tag="mm")
                for kd in range(KD):
                    nc.tensor.matmul(
                        ps,
                        lhsT=w_sb[:, kd, h * HD:(h + 1) * HD],
                        rhs=hT_sb[:, kd, sl],
                        start=(kd == 0),
                        stop=(kd == KD - 1),
                    )
                pending.append((ps, dst_sb, h, qb))

            def v_tile(st_idx):
                flush_rope(keep=1)
                ps = ps_main.tile([P, DC], F32, tag="mm")
                for kd in range(KD):
                    nc.tensor.matmul(
                        ps,
                        lhsT=hT_sb[:, kd, st_idx * P:(st_idx + 1) * P],
                        rhs=wv_sb[:, kd, :],
                        start=(kd == 0),
                        stop=(kd == KD - 1),
                    )
                nc.scalar.activation(v_sb[:, st_idx, :], ps, Copy)

            def proj_units(qb):
                units = []
                for h in range(NHC):
                    units.append(lambda h=h: project(wq_sb, qt_sb, h, qb))
                    units.append(lambda h=h: project(wk_sb, kt_sb, h, qb))
                for s4 in range(4):
                    units.append(lambda s4=s4: v_tile(qb * 4 + s4))
                return units

            def out_proj_unit(qb, s4, tail=False, dve_heavy=False):
                def thunk():
                    st_idx = qb * 4 + s4
                    ost = outstage.tile([P, D], BF16, tag="ost")
                    for eb in range(NQB):
                        ops = ps_main.tile([P, QBS], F32, name="ops", tag="mm")
                        for h in range(NHC):
                            nc.tensor.matmul(
                                ops,
                                lhsT=otb_sb[:, h, st_idx * P:(st_idx + 1) * P],
                                rhs=wo_sb[:, h, eb * QBS:(eb + 1) * QBS],
                                start=(h == 0),
                                stop=(h == NHC - 1),
                            )
                        osl = ost[:, eb * QBS:(eb + 1) * QBS]
                        on_act = (eb % 4 == 0) if dve_heavy else (eb % 2 == 0)
                        if on_act:
                            nc.scalar.activation(osl, ops, Copy)
                        else:
                            nc.vector.tensor_copy(osl, ops)
                        if tail and eb % 2 == 1:
                            # stream the tail out in halves: small final DMA
                            nc.sync.dma_start(
                                out=out[
                                    st_idx * P:(st_idx + 1) * P,
                                    (eb - 1) * QBS:(eb + 1) * QBS,
                                ],
                                in_=ost[:, (eb - 1) * QBS:(eb + 1) * QBS],
                            )
                    if not tail:
                        nc.sync.dma_start(
                            out=out[st_idx * P:(st_idx + 1) * P, :], in_=ost
                        )
                return thunk

            OT_LAG = 3  # P.V matmul trails the score matmul by OT_LAG kt
            # steps so its sem wait is already satisfied and LDWEIGHTS
            # pipelines.

            class AttnBlock:
                """Per-q-block attention state + step emitters.

                For qb > 0 the kt order is non-diagonal tiles first, then the
                diagonal tiles in ascending j: the diagonal score/exp/PV/lane
                work is then sliced to the causally-live columns [j*128:],
                and the denominator lanes are always initialized from a
                full-width (non-diagonal) tile. qb == 0 has only diagonal
                tiles and stays full-width.
                """

                def __init__(self, qb):
                    self.qb = qb
                    self.qsl = slice(qb * QBS, (qb + 1) * QBS)
                    kmax = (qb + 1) * 4
                    if qb == 0:
                        self.kt_order = list(range(4))
                    else:
                        self.kt_order = list(range(0, qb * 4)) + list(
                            range(qb * 4, kmax)
                        )
                    self.pts = {}
                    self.accs = [[None] * 2 for _ in range(NHC)]
                    self.ot_pss = [
                        ps_ot.tile([P, QBS], F32, name="ot_ps", tag="ps_ot")
                        for _ in range(NHC)
                    ]

                def w0(self, kt):
                    j = kt - self.qb * 4
                    if self.qb == 0 or j < 0:
                        return 0
                    return j * P

                def acc_pt(self, h, kt, pt):
                    # h0 lane sums on DVE, h1 lane sums on GpSimd (the Pool
                    # engine is otherwise idle); inits stay on the faster DVE
                    w0 = self.w0(kt)
                    lane = kt % 2
                    if self.accs[h][lane] is None:
                        assert w0 == 0  # lanes init from a full-width tile
                        acc = work.tile(
                            [P, QBS], BF16, name=f"za{h}_{lane}",
                            tag=f"za{h}_{lane}",
                        )
                        nc.vector.tensor_copy(acc, pt)
                        self.accs[h][lane] = acc
                    else:
                        acc = self.accs[h][lane]
                        eng = nc.vector if h == 0 else nc.gpsimd
                        eng.tensor_add(acc[:, w0:], acc[:, w0:], pt[:, w0:])

                def pv_step(self, kt):
                    for h in range(NHC):
                        w0 = self.w0(kt)
                        nc.tensor.matmul(
                            self.ot_pss[h][:, w0:],
                            lhsT=v_sb[:, kt, h * HD:(h + 1) * HD],
                            rhs=self.pts[(h, kt)][:, w0:],
                            start=(kt == self.kt_order[0]),
                            stop=(kt == self.kt_order[-1]),
                            skip_group_check=True,
                        )

                def step(self, ki):
                    qb, kt = self.qb, self.kt_order[ki]
                    w0 = self.w0(kt)
                    for h in range(NHC):
                        sps = ps_main.tile([P, QBS], F32, tag="sps", bufs=3)
                        nc.tensor.matmul(
                            sps[:, w0:],
                            lhsT=kt_sb[:, h, kt * P:(kt + 1) * P],
                            rhs=qt_sb[:, h, qb * QBS + w0:(qb + 1) * QBS],
                            start=True,
                            stop=True,
                        )
                        pt = ptpool.tile([P, QBS], BF16, tag=f"pt{h}")
                        nc.scalar.activation(
                            pt[:, w0:], sps[:, w0:], Exp, scale=SCALE
                        )
                        j = kt - qb * 4
                        if j >= 0:  # diagonal tile: causal mask (on Pool)
                            if qb == 0:
                                w = min(P * (j + 1), QBS)
                                nc.gpsimd.affine_select(
                                    out=pt[:, 0:w],
                                    in_=pt[:, 0:w],
                                    compare_op=mybir.AluOpType.is_ge,
                                    fill=0.0,
                                    base=-kt * P,
                                    pattern=[[1, w]],
                                    channel_multiplier=-1,
                                )
                            else:
                                # only the 128-wide diagonal square is mixed;
                                # everything left of it was sliced away
                                nc.gpsimd.affine_select(
                                    out=pt[:, w0:w0 + P],
                                    in_=pt[:, w0:w0 + P],
                                    compare_op=mybir.AluOpType.is_ge,
                                    fill=0.0,
                                    base=0,
                                    pattern=[[1, P]],
                                    channel_multiplier=-1,
                                )
                        self.pts[(h, kt)] = pt
                        self.acc_pt(h, kt, pt)
                    if ki >= OT_LAG:
                        self.pv_step(self.kt_order[ki - OT_LAG])
                    if ki == len(self.kt_order) - 1:
                        for ki2 in range(max(len(self.kt_order) - OT_LAG, 0),
                                         len(self.kt_order)):
                            self.pv_step(self.kt_order[ki2])

                def den_chain(self):
                    for h in range(NHC):
                        lanes = [a for a in self.accs[h] if a is not None]
                        den_ps = ps_main.tile(
                            [P, QBS], F32, name="den_ps", tag="mm"
                        )
                        for li, lane in enumerate(lanes):
                            nc.tensor.matmul(
                                den_ps, lhsT=ones128, rhs=lane,
                                start=(li == 0), stop=(li == len(lanes) - 1),
                            )
                        # 1/den = exp(-ln(den)) — both funcs live in the SAME
                        # ACT table as the attention Exp (no table reload),
                        # and the DVE reciprocal macro (3.3us!) is avoided.
                        lt = work.tile([P, QBS], F32, tag=f"lt{h}")
                        nc.scalar.activation(lt, den_ps, Ln)
                        r_sb = work.tile([P, QBS], BF16, tag=f"r{h}")
                        nc.scalar.activation(r_sb, lt, Exp, scale=-1.0)
                        nc.vector.tensor_mul(
                            otb_sb[:, h, self.qsl], self.ot_pss[h], r_sb
                        )

            # ---- pipelined emission ---------------------------------------
            # Block order 1,2,3,0: the final attention block is the smallest
            # (4 kt steps), so the kernel tail is short and engine-balanced.
            # Fillers are placed where the attention windows are PE-starved:
            # attn(3) and attn(0) have no projections left, so they get the
            # deferred out-projections; two of out_proj(3)'s tiles are held
            # back to cover den(0)'s latency before the final out_proj(0).
            for u in proj_units(0):
                u()
            for u in proj_units(1):
                u()
            flush_rope()
            plan = [
                (1, lambda: proj_units(2)),
                (2, lambda: proj_units(3)),
                (3, lambda: [out_proj_unit(1, s4) for s4 in range(4)]),
                (0, lambda: [out_proj_unit(2, s4) for s4 in range(4)]
                    + [out_proj_unit(3, s4) for s4 in (0, 1)]),
            ] if NQB == 4 else [
                (qb, (lambda qb=qb: (proj_units(qb + 1) if qb + 1 < NQB else [])
                      + ([out_proj_unit(qb - 1, s4) for s4 in range(4)]
                         if qb > 0 else [])))
                for qb in range(NQB)
            ]
            last_qb = plan[-1][0]
            for qb, fl in plan:
                blk = AttnBlock(qb)
                fillers = fl()
                ns = len(blk.kt_order)
                nf = len(fillers)
                fi = 0
                for ki in range(ns):
                    blk.step(ki)
                    tgt = (ki + 1) * nf // ns
                    while fi < tgt:
                        fillers[fi]()
                        fi += 1
                while fi < nf:
                    fillers[fi]()
                    fi += 1
                flush_rope()
                blk.den_chain()

            if NQB == 4:
                for s4 in (2, 3):  # cover den(0) latency
                    out_proj_unit(3, s4)()
            for s4 in range(4):
                out_proj_unit(last_qb, s4, tail=True, dve_heavy=True)()
    if split_waits:
        _split_excess_waits(nc)
    return nc


_NC_CACHE = {}


def _get_nc():
    if "nc" not in _NC_CACHE:
        _NC_CACHE["nc"] = build_nc()
    return _NC_CACHE["nc"]


def _rotation_matrix_T():
    # rot(x)[2i] = -x[2i+1]; rot(x)[2i+1] = x[2i].  R[i,j] coefficient of x[j].
    R = np.zeros((HD, HD), np.float32)
    idx = np.arange(0, HD, 2)
    R[idx, idx + 1] = -1.0
    R[idx + 1, idx] = 1.0
    return np.ascontiguousarray(R.T)


def prepare_in_maps(hidden_states, sin, cos, Wq, Wk, Wv, Wo):
    hidden_states = np.asarray(hidden_states, dtype=np.float32)
    sin = np.asarray(sin, dtype=np.float32)
    cos = np.asarray(cos, dtype=np.float32)
    Wq = np.asarray(Wq, dtype=np.float32)
    Wk = np.asarray(Wk, dtype=np.float32)
    Wv = np.asarray(Wv, dtype=np.float32)
    Wo = np.asarray(Wo, dtype=np.float32)

    hT = np.ascontiguousarray(hidden_states[0].T).astype(NPBF16)
    ct = np.ascontiguousarray(np.repeat(cos, 2, axis=1).T).astype(NPBF16)
    st = np.ascontiguousarray(np.repeat(sin, 2, axis=1).T).astype(NPBF16)
    rot = _rotation_matrix_T().astype(NPBF16)

    in_maps = []
    for c in range(N_CORES):
        e0 = c * DC
        in_maps.append(
            {
                "hT": hT,
                "wq": np.ascontiguousarray(Wq[e0:e0 + DC, :].T).astype(NPBF16),
                "wk": np.ascontiguousarray(Wk[e0:e0 + DC, :].T).astype(NPBF16),
                "wv": np.ascontiguousarray(Wv[e0:e0 + DC, :].T).astype(NPBF16),
                "wo": np.ascontiguousarray(Wo[:, e0:e0 + DC].T).astype(NPBF16),
                "ct": ct,
                "st": st,
                "rot": rot,
            }
        )
    return in_maps


def kernel(hidden_states, attention_mask, sin, cos, Wq, Wk, Wv, Wo):
    in_maps = prepare_in_maps(hidden_states, sin, cos, Wq, Wk, Wv, Wo)
    nc = _get_nc()
    res = run_bass_kernel_spmd(nc, in_maps, list(range(N_CORES)))
    out = res.results[0]["out"].astype(np.float32)
    for c in range(1, N_CORES):
        out += res.results[c]["out"].astype(np.float32)
    return out[None]
